# revision 18
# baseline (speedup 1.0000x reference)
"""Trainium2 Bass kernel for nn_AdvisorCrossAttentionAdapter.

Data-parallel over batch: core c computes batch c end-to-end (B=8 = n_cores).

Algebraic restructuring (validated vs the reference in fp32 numpy):
  scores = hidden @ G @ trip0^T  with G = Wq^T Wk / sqrt(H)  (host weight
  folding), so the S x H q-projection (4.3 GF) becomes a T x H one (1.1 GF).
  out = attn @ (v_final @ Wo^T): the out-projection is applied to the T-row
  v_final instead of the S-row context (another 3.2 GF saved).
  Logic-gate value selection decomposed into relus:
    d = v1-v2, s = v1+v2, r1 = relu(d), r2 = relu(-d), rs2 = relu(-s)
    v_final = c_v1*v1 + c_v2*v2 + c_r1*r1 + c_r2*r2 + c_rs2*rs2 + c_vrel*vrel
  with per-t coefficients in {-1,0,1} (host-computed from advisor_ids) applied
  via block-diagonal matmuls that also transpose [t,h] -> [h,t] for free.
  Softmax runs without max subtraction (scores ~ N(0,1), exp < 3e3 << fp16
  max); normalization is deferred to the final output drain.

On-chip operand dtype is fp16 (e5m10: all value ranges here are O(1..3e3),
so fp16 keeps ~5e-4 relative precision vs bf16's 4e-3) with fp32 PSUM
accumulation. Matmul order maximizes stationary-weight reuse, and phases are
ordered so PE never waits on DMA: kMT -> scoresT(+exp) -> v -> blend -> w ->
out.
"""

import math

import numpy as np

N_CORES = 8
B, S, H, L = 8, 2048, 1024, 1536
T = L // 3            # 512
NT = T // 128         # 4 t-tiles
NH = H // 128         # 8 h-tiles
SCHUNK = 512
NSC = S // SCHUNK     # 4 s-chunks
NST = SCHUNK // 128   # 4 s-subtiles per chunk

_CACHE = {}
DT16 = "f16"  # "f16" or "bf16" for on-chip 2-byte operands


def _split_excess_waits(nc, mybir, lim_default=1):
    """Walrus in this container rejects instructions with too many sync
    waits. Move excess waits onto InstEventSemaphore carriers inserted just
    before the offender (same engine, same block): engine-local order is
    preserved so semantics are identical."""
    f = nc.m.functions[0]
    for b in f.blocks:
        insts = b.instructions
        i = 0
        while i < len(insts):
            ins = insts[i]
            si = ins.sync_info
            nm = type(ins).__name__
            lim = 1 if nm in ("InstDrain", "InstNoOp") else lim_default
            if si is not None and si.on_wait and len(si.on_wait) > lim:
                waits = list(si.on_wait)
                extra, keep = waits[:-lim], waits[-lim:]
                ins.sync_info = mybir.SyncInfo(on_wait=keep, on_update=si.on_update)
                for w in extra:
                    e = mybir.InstEventSemaphore(
                        name=nc.get_next_instruction_name(), ins=[], outs=[])
                    e.engine = ins.engine
                    e.sync_info = mybir.SyncInfo(on_wait=[w], on_update=[])
                    insts.insert(i, e)
                    i += 1
            i += 1


def build_program(reps=1):
    import concourse.bass as bass
    import concourse.mybir as mybir
    from contextlib import ExitStack
    from concourse.tile import TileContext

    f16 = mybir.dt.float16 if DT16 == "f16" else mybir.dt.bfloat16
    f32 = mybir.dt.float32

    nc = bass.Bass("TRN2", target_bir_lowering=False, debug=False,
                   num_devices=N_CORES)

    hT_d = nc.declare_dram_parameter("hT", [H, S], f16, isOutput=False)
    aT_d = [nc.declare_dram_parameter(f"aT{k}", [H, T], f16, isOutput=False)
            for k in range(3)]
    Gt_d = nc.declare_dram_parameter("Gt", [H, H], f16, isOutput=False)
    WvT_d = nc.declare_dram_parameter("WvT", [H, H], f16, isOutput=False)
    WoT_d = nc.declare_dram_parameter("WoT", [H, H], f16, isOutput=False)
    diag_d = nc.declare_dram_parameter("diag", [128, 6 * NT * 128], f16,
                                       isOutput=False)
    out_d = nc.declare_dram_parameter("out", [S, H], f32, isOutput=True)

    with TileContext(nc) as tc:
        for _rep in range(reps):
            with ExitStack() as ctx:
                _emit_body(nc, tc, ctx, mybir, hT_d, aT_d, Gt_d, WvT_d, WoT_d,
                           diag_d, out_d)

    _split_excess_waits(nc, mybir)
    return nc


def _emit_body(nc, tc, ctx, mybir, hT_d, aT_d, Gt_d, WvT_d, WoT_d, diag_d,
               out_d):
    from contextlib import ExitStack

    f16 = mybir.dt.float16 if DT16 == "f16" else mybir.dt.bfloat16
    f32 = mybir.dt.float32
    ACT = mybir.ActivationFunctionType
    ALU = mybir.AluOpType

    pconst = ctx.enter_context(tc.tile_pool(name="pconst", bufs=1))
    ones_f = pconst.tile([128, 1], f32, tag="ones_f", name="ones_f")
    nc.vector.memset(ones_f[:], 1.0)
    ones = pconst.tile([128, 1], f16, tag="ones", name="ones")
    nc.vector.tensor_copy(out=ones[:], in_=ones_f[:])
    warm = pconst.tile([128, 1], f32, tag="warm", name="warm")
    nc.scalar.activation(warm[:], ones_f[:], ACT.Exp)  # pin exp table set
    diag_sb = pconst.tile([128, 6 * NT * 128], f16, tag="diag", name="diag_sb")
    kMT_sb = [pconst.tile([128, T], f16, tag=f"kMT{i}", name=f"kMT{i}")
              for i in range(NH)]
    vfT_sb = [pconst.tile([128, T], f16, tag=f"vfT{i}", name=f"vfT{i}")
              for i in range(NH)]
    w_sb = [pconst.tile([128, H], f16, tag=f"wsb{i}", name=f"wsb{i}")
            for i in range(NT)]
    # exps[tt][sc]: exp(scores^T) tiles [t'=128, s-chunk=512]
    exps = [[pconst.tile([128, SCHUNK], f16, tag=f"exp{tt}_{sc}",
                         name=f"exp{tt}_{sc}") for sc in range(NSC)]
            for tt in range(NT)]
    recip = pconst.tile([128, S // 128], f32, tag="recip", name="recip")

    # ---------------- phase A: kMT, scoresT + exp, denominators -----------
    pa = ctx.enter_context(tc.tile_pool(name="pa", bufs=1))
    pw = ctx.enter_context(tc.tile_pool(name="pw", bufs=24))
    with ExitStack() as phA:
        pht = phA.enter_context(tc.tile_pool(name="pht", bufs=1))

        gt, a0 = [], []
        for i in range(NH):
            t = pw.tile([128, H], f16, tag="w", name="wslot")
            nc.sync.dma_start(out=t[:], in_=Gt_d[i * 128:(i + 1) * 128, :])
            gt.append(t)
            t = pa.tile([128, T], f16, tag=f"a0_{i}", name=f"a0_{i}")
            nc.sync.dma_start(out=t[:], in_=aT_d[0][i * 128:(i + 1) * 128, :])
            a0.append(t)
        hts = []
        for i in range(NH):
            t = pht.tile([128, S], f16, tag=f"h{i}", name=f"h{i}")
            nc.sync.dma_start(out=t[:], in_=hT_d[i * 128:(i + 1) * 128, :])
            hts.append(t)
        a1, a2 = [], []
        for k, lst in ((1, a1), (2, a2)):
            for i in range(NH):
                t = pa.tile([128, T], f16, tag=f"a{k}_{i}", name=f"a{k}_{i}")
                nc.sync.dma_start(out=t[:],
                                  in_=aT_d[k][i * 128:(i + 1) * 128, :])
                lst.append(t)
        # pre-add the triplet slots so phase B projects d/s/vrel directly
        a1p, a1m = [], []
        for i in range(NH):
            tp = pa.tile([128, T], f16, tag=f"a1p_{i}", name=f"a1p_{i}")
            nc.vector.tensor_add(out=tp[:], in0=a1[i][:], in1=a2[i][:])
            a1p.append(tp)
            tm = pa.tile([128, T], f16, tag=f"a1m_{i}", name=f"a1m_{i}")
            nc.vector.tensor_sub(out=tm[:], in0=a1[i][:], in1=a2[i][:])
            a1m.append(tm)

        # prefetch phase B/C weights now (pw has slots for all 3 sets)
        wv, wo = [], []
        for i in range(NH):
            t = pw.tile([128, H], f16, tag="w", name="wslot")
            nc.sync.dma_start(out=t[:], in_=WvT_d[i * 128:(i + 1) * 128, :])
            wv.append(t)
        for i in range(NH):
            t = pw.tile([128, H], f16, tag="w", name="wslot")
            nc.sync.dma_start(out=t[:], in_=WoT_d[i * 128:(i + 1) * 128, :])
            wo.append(t)
        nc.sync.dma_start(out=diag_sb[:], in_=diag_d[:, :])

        # kMT[h,t] = sum_h' Gt[h',h] * aT0[h',t]; k-outer so PE starts as
        # soon as the first Gt/aT0 tiles land
        with tc.tile_pool(name="ppk", bufs=4, space="PSUM") as ppk:
            for g in range(2):
                pss = [ppk.tile([128, T], f32, tag="pk", name="pk")
                       for _ in range(4)]
                for kh in range(NH):
                    for j in range(4):
                        mh = g * 4 + j
                        nc.tensor.matmul(
                            pss[j][:],
                            lhsT=gt[kh][:, mh * 128:(mh + 1) * 128],
                            rhs=a0[kh][:],
                            start=(kh == 0), stop=(kh == NH - 1))
                for j in range(4):
                    nc.vector.tensor_copy(out=kMT_sb[g * 4 + j][:],
                                          in_=pss[j][:])

        # scoresT + exp; lhsT (kMT block) reused across the 4 s-chunks
        with tc.tile_pool(name="psps", bufs=4, space="PSUM") as psps:
            for tt in range(NT):
                pss = [psps.tile([128, SCHUNK], f32, tag="sps", name="sps")
                       for _ in range(NSC)]
                for kh in range(NH):
                    for sc in range(NSC):
                        nc.tensor.matmul(
                            pss[sc][:],
                            lhsT=kMT_sb[kh][:, tt * 128:(tt + 1) * 128],
                            rhs=hts[kh][:, sc * SCHUNK:(sc + 1) * SCHUNK],
                            start=(kh == 0), stop=(kh == NH - 1))
                for sc in range(NSC):
                    nc.scalar.activation(exps[tt][sc][:], pss[sc][:], ACT.Exp)


    # ---------------- phase B: v-projections + gate blend -----------------
    with ExitStack() as phB:
        psrc = phB.enter_context(tc.tile_pool(name="psrc", bufs=1))
        with tc.tile_pool(name="pvps", bufs=3, space="PSUM") as pvps, \
             tc.tile_pool(name="pbps", bufs=2, space="PSUM") as pbps:
            for tt in range(NT):
                tsl = slice(tt * 128, (tt + 1) * 128)
                pvs = {}
                for name, asrc in (("s", a1p), ("d", a1m), ("vr", a0)):
                    ps = pvps.tile([128, H], f32, tag="vps", name="vps")
                    for kh in range(NH):
                        for oh in range(2):
                            nc.tensor.matmul(
                                ps[:, oh * 512:(oh + 1) * 512],
                                lhsT=asrc[kh][:, tsl],
                                rhs=wv[kh][:, oh * 512:(oh + 1) * 512],
                                start=(kh == 0), stop=(kh == NH - 1))
                    pvs[name] = ps
                ss = psrc.tile([128, H], f16, tag="ss", name="ss")
                nc.vector.tensor_copy(out=ss[:], in_=pvs["s"][:])
                sd = psrc.tile([128, H], f16, tag="sd", name="sd")
                nc.vector.tensor_copy(out=sd[:], in_=pvs["d"][:])
                vrs = psrc.tile([128, H], f16, tag="vrs", name="vrs")
                nc.scalar.activation(vrs[:], pvs["vr"][:], ACT.Copy)
                r1 = psrc.tile([128, H], f16, tag="r1", name="r1")
                nc.scalar.activation(r1[:], pvs["d"][:], ACT.Relu)
                r2 = psrc.tile([128, H], f16, tag="r2", name="r2")
                nc.scalar.activation(r2[:], pvs["d"][:], ACT.Relu, scale=-1.0)
                rs2 = psrc.tile([128, H], f16, tag="rs2", name="rs2")
                nc.scalar.activation(rs2[:], pvs["s"][:], ACT.Relu, scale=-1.0)

                srcs = [(ss, 0), (sd, 1), (r1, 2), (r2, 3), (rs2, 4),
                        (vrs, 5)]
                for hh in range(NH):
                    bps = pbps.tile([128, 128], f32, tag="bps", name="bps")
                    for j, (srct, i) in enumerate(srcs):
                        dcol = (i * NT + tt) * 128
                        nc.tensor.matmul(
                            bps[:],
                            lhsT=srct[:, hh * 128:(hh + 1) * 128],
                            rhs=diag_sb[:, dcol:dcol + 128],
                            start=(j == 0), stop=(j == len(srcs) - 1))
                    nc.vector.tensor_copy(out=vfT_sb[hh][:, tsl], in_=bps[:])

    # ---------------- phase C: w = v_final @ WoT --------------------------
    with tc.tile_pool(name="pwps", bufs=2, space="PSUM") as pwps:
        for tt in range(NT):
            ps = pwps.tile([128, H], f32, tag="wps", name="wps")
            for kh in range(NH):
                for oh in range(2):
                    nc.tensor.matmul(
                        ps[:, oh * 512:(oh + 1) * 512],
                        lhsT=vfT_sb[kh][:, tt * 128:(tt + 1) * 128],
                        rhs=wo[kh][:, oh * 512:(oh + 1) * 512],
                        start=(kh == 0), stop=(kh == NH - 1))
            if tt % 2 == 0:
                nc.scalar.activation(w_sb[tt][:], ps[:], ACT.Copy)
            else:
                nc.vector.tensor_copy(out=w_sb[tt][:], in_=ps[:])

        # denominators -> per-partition reciprocal columns
    with tc.tile_pool(name="pdps", bufs=2, space="PSUM") as pdps, \
         tc.tile_pool(name="prct", bufs=2, space="PSUM") as prct, \
         tc.tile_pool(name="pdrow", bufs=2) as pdrow:
        for sc in range(NSC):
            dps = pdps.tile([1, SCHUNK], f32, tag="dps", name="dps")
            for tt in range(NT):
                nc.tensor.matmul(dps[:], lhsT=ones[:], rhs=exps[tt][sc][:],
                                 start=(tt == 0), stop=(tt == NT - 1))
            drow = pdrow.tile([1, SCHUNK], f32, tag="drow", name="drow")
            nc.vector.tensor_copy(out=drow[:], in_=dps[:])
            rct = prct.tile([128, NST], f32, tag="rct", name="rct")
            for j in range(NST):
                nc.tensor.matmul(rct[:, j:j + 1],
                                 lhsT=drow[0:1, j * 128:(j + 1) * 128],
                                 rhs=ones_f[0:1, 0:1],
                                 start=True, stop=True)
            nc.vector.reciprocal(out=recip[:, sc * NST:(sc + 1) * NST],
                                 in_=rct[:])

    # ---------------- phase D: out = attn @ w, normalized -----------------
    with tc.tile_pool(name="pout", bufs=4) as pout, \
         tc.tile_pool(name="pops", bufs=6, space="PSUM") as pops:
        for s_idx in range(S // 128):
            sc, st = divmod(s_idx, NST)
            outp = pout.tile([128, H], f32, tag="outp", name="outp")
            for oh in range(2):
                ps = pops.tile([128, 512], f32, tag="ops", name="ops")
                for tt in range(NT):
                    nc.tensor.matmul(
                        ps[:],
                        lhsT=exps[tt][sc][:, st * 128:(st + 1) * 128],
                        rhs=w_sb[tt][:, oh * 512:(oh + 1) * 512],
                        start=(tt == 0), stop=(tt == NT - 1))
                osl = slice(oh * 512, (oh + 1) * 512)
                if oh == 0:
                    nc.vector.tensor_scalar(
                        out=outp[:, osl], in0=ps[:],
                        scalar1=recip[:, s_idx:s_idx + 1], scalar2=None,
                        op0=ALU.mult)
                else:
                    nc.scalar.activation(
                        outp[:, osl], ps[:], ACT.Copy,
                        scale=recip[:, s_idx:s_idx + 1])
            nc.sync.dma_start(out=out_d[s_idx * 128:(s_idx + 1) * 128, :],
                              in_=outp[:])


def prepare_inputs(hidden_states, advisor_states, advisor_ids, Wq, Wk, Wv, Wo):
    """Host-side sharding + layout prep. Returns per-core input maps."""
    if DT16 == "f16":
        np16 = np.float16
    else:
        import ml_dtypes
        np16 = ml_dtypes.bfloat16
    hidden_states = np.asarray(hidden_states, dtype=np.float32)
    advisor_states = np.asarray(advisor_states, dtype=np.float32)
    advisor_ids = np.asarray(advisor_ids)
    Wq = np.asarray(Wq, dtype=np.float32)
    Wk = np.asarray(Wk, dtype=np.float32)
    Wv = np.asarray(Wv, dtype=np.float32)
    Wo = np.asarray(Wo, dtype=np.float32)

    trip = advisor_states.reshape(B, T, 3, H)
    rel = advisor_ids.reshape(B, T, 3)[:, :, 0]
    m = [(rel == i).astype(np.float32) for i in range(5)]
    m5 = (rel >= 5).astype(np.float32)
    c_v1 = m[0] - m[2]
    c_v2 = m[1] + m[3]
    coeffs = np.stack([
        (c_v1 + c_v2) / 2,    # s = v1+v2 (projected directly)
        (c_v1 - c_v2) / 2,    # d = v1-v2 (projected directly)
        m[1] - m[0] + m[4],   # r1 = relu(d)
        m[4],                 # r2 = relu(-d)
        m[3],                 # rs2 = relu(-s)
        m5,                   # vrel
    ], axis=1)                # [B, 6, T]

    # diag[b, p, (i*NT+tt)*128 + q] = coeffs[b, i, tt*128+p] * (p == q)
    diag = np.zeros((B, 128, 6 * NT * 128), np16)
    pidx = np.arange(128)
    for i in range(6):
        for tt in range(NT):
            col = (i * NT + tt) * 128
            diag[:, pidx, col + pidx] = coeffs[:, i, tt * 128 + pidx]

    Gt = (Wk.astype(np.float64).T @ Wq.astype(np.float64)
          / math.sqrt(H)).astype(np16)
    WvT = np.ascontiguousarray(Wv.T).astype(np16)
    WoT = np.ascontiguousarray(Wo.T).astype(np16)

    hT = np.ascontiguousarray(
        hidden_states.transpose(0, 2, 1)).astype(np16)       # [B,H,S]
    aT = [np.ascontiguousarray(
        trip[:, :, k, :].transpose(0, 2, 1)).astype(np16)
          for k in range(3)]                                       # [B,H,T]

    in_maps = []
    for c in range(N_CORES):
        in_maps.append({
            "hT": hT[c], "aT0": aT[0][c], "aT1": aT[1][c], "aT2": aT[2][c],
            "Gt": Gt, "WvT": WvT, "WoT": WoT, "diag": diag[c],
        })
    return in_maps


def kernel(hidden_states, advisor_states, advisor_ids, Wq, Wk, Wv, Wo):
    from concourse.bass_utils import run_bass_kernel_spmd

    if "nc" not in _CACHE:
        _CACHE["nc"] = build_program()
    nc = _CACHE["nc"]

    in_maps = prepare_inputs(hidden_states, advisor_states, advisor_ids,
                             Wq, Wk, Wv, Wo)
    res = run_bass_kernel_spmd(nc, in_maps, list(range(N_CORES)))
    out = np.stack([res.results[c]["out"] for c in range(N_CORES)], axis=0)
    return out.astype(np.float32)
